# revision 23
# baseline (speedup 1.0000x reference)
"""Masked dot-product attention (B=16, Nq=Nkv=2048, d=64) on 8 TRN2 NeuronCores.

Strategy (per core: 2 batch slots, data-parallel over B):
  - Host pre-transposes Q/K to [64, N] per batch (input prep, like the mask),
    so Q^T/K^T tiles DMA straight into SBUF. Row 64 of Q^T is all-ones, row
    64 of K^T is the additive mask (0 / -1e6): the QK matmul computes masked
    scores directly in BOTH orientations (f32r, 1 cyc/row).
  - S^T tiles [k=128, q=1024] (two matmuls sharing lhsT into one 2-bank psum)
    -> one ACT exp (scale=1/8) -> E^T f32r, feeding the AV matmul
    (lhsT = V1 = V with ones column -> row 64 of out^T = softmax denominator).
  - out^T [65, 512] PE-transposed to [q, 65]; r = 1/den (DVE reciprocal);
    out = out' * r (DVE tensor_scalar).
  - attn tiles computed DIRECTLY in [q, k] layout: S = matmul(lhsT=Q^T slice,
    rhs=K^T chunk) -> ACT exp -> SBUF; DVE tensor_scalar_mul by r into attn
    row buffers -> SWDGE DMA. Only Exp on ACT (single table load).
  - Sparsity: slot0 computes t0 k-tiles of 128, slot1 t1 (valid_len rounded
    up to 128); host pairs batches (largest with smallest valid_len) and
    splits the 8 cores into two groups of 4 with their own compiled (t0, t1)
    programs, dispatched concurrently on disjoint device subsets. attn
    columns beyond the computed range stay zero (outputs are pre-zeroed
    device buffers). PE runs at a fixed 1.2 GHz on this part (HAM never
    ungates, even for bf16 streams), so the design minimizes PE cycles:
    f32r everywhere (1 cyc/row at N>=256), no on-device transposes of E.
"""
import numpy as np
import concourse.bacc as bacc
import concourse.mybir as mybir
import concourse.tile as tile
from concourse.bass_utils import run_bass_kernel_spmd
from contextlib import ExitStack

dt = mybir.dt
F32 = dt.float32
F32R = dt.float32r

B, N, D = 16, 2048, 64
NCORES = 8
BPC = B // NCORES
NEG = -1e6
SCALE = 1.0 / 8.0

_NC_CACHE = {}


def _build(ktile_counts):
    """ktile_counts: tuple of BPC ints, #k-tiles (of 128) computed per slot."""
    nc = bacc.Bacc(None, target_bir_lowering=False)
    qt_d = nc.dram_tensor("qT", [BPC, D, N], F32, kind="ExternalInput")
    kt_d = nc.dram_tensor("kT", [BPC, D, N], F32, kind="ExternalInput")
    v1_d = nc.dram_tensor("v1", [BPC, N, D + 1], F32, kind="ExternalInput")
    mb_d = nc.dram_tensor("mb", [BPC, N], F32, kind="ExternalInput")
    ones_d = nc.dram_tensor("ones", [N], F32, kind="ExternalInput")
    iden_d = nc.dram_tensor("iden", [D + 1, D + 1], F32, kind="ExternalInput")
    attn_d = nc.dram_tensor("attn", [BPC, N, N], F32, kind="ExternalOutput")
    out_d = nc.dram_tensor("out", [BPC, N, D], F32, kind="ExternalOutput")

    Exp = mybir.ActivationFunctionType.Exp
    maxt = max(ktile_counts)

    def kchunks(t):
        """split t k-tiles into chunks of <=4 tiles: [(tile0, ntiles), ...]"""
        out = []
        k = 0
        while k < t:
            n = min(4, t - k)
            out.append((k, n))
            k += n
        return out

    with tile.TileContext(nc) as tc, ExitStack() as ctx:
        const_p = ctx.enter_context(tc.tile_pool(name="const", bufs=1))
        kt_p = ctx.enter_context(tc.tile_pool(name="ktp", bufs=2 * 4))
        qt_p = ctx.enter_context(tc.tile_pool(name="qtp", bufs=2 * 2))
        v_p = ctx.enter_context(tc.tile_pool(name="vp", bufs=2))
        et_p = ctx.enter_context(tc.tile_pool(name="etp", bufs=16))
        e2_p = ctx.enter_context(tc.tile_pool(name="e2p", bufs=20))
        ot_p = ctx.enter_context(tc.tile_pool(name="otp", bufs=4))
        r_p = ctx.enter_context(tc.tile_pool(name="rp", bufs=16))
        o_p = ctx.enter_context(tc.tile_pool(name="op", bufs=4))
        arow_p = ctx.enter_context(tc.tile_pool(name="arowp", bufs=4))

        ps_s = ctx.enter_context(tc.tile_pool(name="ps_s", bufs=2, space="PSUM"))
        ps_s2 = ctx.enter_context(tc.tile_pool(name="ps_s2", bufs=2, space="PSUM"))
        ps_av = ctx.enter_context(tc.tile_pool(name="ps_av", bufs=1, space="PSUM"))
        ps_m = ctx.enter_context(tc.tile_pool(name="ps_m", bufs=1, space="PSUM"))

        iden = const_p.tile([D + 1, D + 1], F32)
        nc.sync.dma_start(iden[:], iden_d[:, :])

        def attn_exp_phase(qc, KTc, QTh, t):
            """S direct + exp -> e2 tiles for all 4 q-subtiles of chunk qc.
            Emitted BEFORE the AV chain so PE/ACT have bridge work while AV
            waits on exp1 stragglers. Returns e2 tiles for the mul phase."""
            QT = QTh[qc // 2]
            g = (qc % 2) * 512
            e2s = {}
            for qs in range(4):
                g0 = g + qs * 128
                for kc, (t0, nt) in enumerate(kchunks(t)):
                    w = 128 * nt
                    s2 = ps_s2.tile([128, 512], F32, tag="ps_s2")
                    nc.tensor.matmul(
                        s2[:, 0:w], QT[:, g0:g0 + 128], KTc[kc][:, 0:w],
                        start=True, stop=True,
                    )
                    e2 = e2_p.tile([128, 512], F32, tag="e2")
                    nc.scalar.activation(e2[:, 0:w], s2[:, 0:w], Exp, scale=SCALE)
                    e2s[(qs, kc)] = e2
            return e2s

        def attn_mul_phase(b, qc, e2s, rs, t, kw):
            """r-mul the e2 tiles into attn row buffers and DMA them out."""
            for qs in range(4):
                arow = arow_p.tile([128, 128 * maxt], F32, tag="arow")
                for kc, (t0, nt) in enumerate(kchunks(t)):
                    w = 128 * nt
                    nc.vector.tensor_scalar_mul(
                        arow[:, t0 * 128:t0 * 128 + w], e2s[(qs, kc)][:, 0:w],
                        rs[qs][:, 0:1],
                    )
                row0 = qc * 512 + qs * 128
                nc.gpsimd.dma_start(attn_d[b, row0:row0 + 128, 0:kw], arow[:, 0:kw])

        def emit_loads(b):
            t = ktile_counts[b]
            kw = 128 * t
            KTc = []
            QTh = []

            def load_kc(c):
                t0, nt = kchunks(t)[c]
                w = 128 * nt
                KT = kt_p.tile([65, 512], F32R, tag="ktc")
                nc.sync.dma_start(
                    KT[64:65, 0:w], mb_d[b, t0 * 128:t0 * 128 + w].bitcast(F32R)
                )
                nc.sync.dma_start(
                    KT[0:D, 0:w], kt_d[b, :, t0 * 128:t0 * 128 + w].bitcast(F32R)
                )
                KTc.append(KT)

            def load_qh(h):
                QT = qt_p.tile([65, 1024], F32R, tag="qth")
                nc.sync.dma_start(
                    QT[64:65, :], ones_d[h * 1024:(h + 1) * 1024].bitcast(F32R)
                )
                nc.sync.dma_start(
                    QT[0:D, :], qt_d[b, :, h * 1024:(h + 1) * 1024].bitcast(F32R)
                )
                QTh.append(QT)

            load_kc(0)
            load_qh(0)
            V1s = v_p.tile([128, maxt, D + 1], F32R, tag="v1s")
            nc.sync.dma_start(
                V1s[:, 0:t, :],
                v1_d[b, 0:kw, :].rearrange("(t p) d -> p t d", p=128).bitcast(F32R),
            )
            for c in range(1, len(kchunks(t))):
                load_kc(c)
            load_qh(1)
            return KTc, QTh, V1s

        def emit_compute(b, qp, KTc, QTh, V1s):
            t = ktile_counts[b]
            nkt = t
            kw = 128 * t
            QT = QTh[qp]
            ets = []
            for kt in range(nkt):
                sp = ps_s.tile([128, 1024], F32, tag="ps_s")
                lhs = KTc[kt // 4][:, (kt % 4) * 128:((kt % 4) + 1) * 128]
                nc.tensor.matmul(
                    sp[:, 0:512], lhs, QT[:, 0:512], start=True, stop=True
                )
                nc.tensor.matmul(
                    sp[:, 512:1024], lhs, QT[:, 512:1024], start=True, stop=True
                )
                et = et_p.tile([128, 1024], F32R, tag="et")
                nc.scalar.activation(et[:], sp[:], Exp, scale=SCALE)
                ets.append(et)

            for half in range(2):
                qc = qp * 2 + half
                h0 = half * 512
                e2s = attn_exp_phase(qc, KTc, QTh, t)
                avp = ps_av.tile([D + 1, 512], F32, tag="ps_av")
                for kt in range(nkt):
                    nc.tensor.matmul(
                        avp[:], V1s[:, kt, :], ets[kt][:, h0:h0 + 512],
                        start=(kt == 0), stop=(kt == nkt - 1),
                    )
                oT = ot_p.tile([D + 1, 512], F32, tag="ot")
                nc.vector.tensor_copy(oT[:], avp[:])

                rs = []
                for qs in range(4):
                    otp = ps_m.tile([128, D + 1], F32, tag="psm")
                    nc.tensor.transpose(
                        otp[:], oT[:, qs * 128:(qs + 1) * 128], iden[:]
                    )
                    r = r_p.tile([128, 1], F32, tag="r")
                    nc.vector.reciprocal(r[:], otp[:, D:D + 1])
                    osb = o_p.tile([128, D], F32, tag="o")
                    nc.vector.tensor_scalar_mul(osb[:], otp[:, 0:D], r[:, 0:1])
                    nc.sync.dma_start(
                        out_d[b, qc * 512 + qs * 128: qc * 512 + (qs + 1) * 128, :],
                        osb[:],
                    )
                    rs.append(r)

                attn_mul_phase(b, qc, e2s, rs, t, kw)

        # small slot (b=1) opens and closes the kernel: tiny first loads get
        # PE started early, tiny last attn rows keep the tail flush short
        l1 = emit_loads(1)
        emit_compute(1, 0, *l1)
        l0 = emit_loads(0)
        emit_compute(0, 0, *l0)
        emit_compute(0, 1, *l0)
        emit_compute(1, 1, *l1)

    nc.finalize()
    return nc


def _get_nc(chunk_counts):
    key = tuple(chunk_counts)
    if key not in _NC_CACHE:
        _NC_CACHE[key] = _build(key)
    return _NC_CACHE[key]


def _dispatch_group(nc, in_maps, devices):
    """Mirror of bass2jax.run_bass_via_pjrt's multi-core branch, but on an
    explicit device subset. Returns (lazy out_arrs, out_names, out_avals);
    caller materializes with np.asarray after dispatching all groups."""
    import jax
    from jax.experimental.shard_map import shard_map
    from jax.sharding import Mesh, PartitionSpec
    import concourse.mybir as mybir
    from concourse.bass2jax import (
        _bass_exec_p, install_neuronx_cc_hook, partition_id_tensor,
    )

    install_neuronx_cc_hook()

    partition_name = nc.partition_id_tensor.name if nc.partition_id_tensor else None
    in_names, out_names, out_avals, zero_outs = [], [], [], []
    for alloc in nc.m.functions[0].allocations:
        if not isinstance(alloc, mybir.MemoryLocationSet):
            continue
        name = alloc.memorylocations[0].name
        if alloc.kind == "ExternalInput":
            if name != partition_name:
                in_names.append(name)
        elif alloc.kind == "ExternalOutput":
            out_names.append(name)
            shape = tuple(alloc.tensor_shape)
            dtype = mybir.dt.np(alloc.dtype)
            out_avals.append(jax.core.ShapedArray(shape, dtype))
            zero_outs.append(np.zeros(shape, dtype))
    n_params = len(in_names)
    n_outs = len(out_avals)
    all_names = in_names + out_names
    if partition_name is not None:
        all_names.append(partition_name)
    donate = tuple(range(n_params, n_params + n_outs))

    def _body(*args):
        operands = list(args)
        if partition_name is not None:
            operands.append(partition_id_tensor())
        outs = _bass_exec_p.bind(
            *operands,
            out_avals=tuple(out_avals),
            in_names=tuple(all_names),
            out_names=tuple(out_names),
            lowering_input_output_aliases=(),
            sim_require_finite=True,
            sim_require_nnan=True,
            nc=nc,
        )
        return tuple(outs)

    n_cores = len(devices)
    mesh = Mesh(np.asarray(devices), ("core",))
    in_specs = (PartitionSpec("core"),) * (n_params + n_outs)
    out_specs = (PartitionSpec("core"),) * n_outs
    sharded = jax.jit(
        shard_map(_body, mesh=mesh, in_specs=in_specs, out_specs=out_specs,
                  check_rep=False),
        donate_argnums=donate, keep_unused=True,
    )
    concat_in = [
        np.concatenate([np.asarray(in_maps[c][in_names[i]]) for c in range(n_cores)],
                       axis=0)
        for i in range(n_params)
    ]
    concat_zeros = [
        np.zeros((n_cores * z.shape[0], *z.shape[1:]), z.dtype) for z in zero_outs
    ]
    out_arrs = sharded(*concat_in, *concat_zeros)
    return out_arrs, out_names, out_avals


def _run_groups(group_ncs, group_maps):
    """Dispatch each group's program on its device subset, then materialize.
    group_ncs: list of nc; group_maps: list of list-of-in_map (4 cores each)."""
    import jax
    devices = jax.devices()
    results = []
    k = 0
    lazy = []
    for nc, maps in zip(group_ncs, group_maps):
        devs = devices[k:k + len(maps)]
        lazy.append(_dispatch_group(nc, maps, devs))
        k += len(maps)
    for (out_arrs, out_names, out_avals), maps in zip(lazy, group_maps):
        n_cores = len(maps)
        per_core = [
            {name: np.asarray(out_arrs[i]).reshape(n_cores, *out_avals[i].shape)[c]
             for i, name in enumerate(out_names)}
            for c in range(n_cores)
        ]
        results.extend(per_core)
    return results


def _shard(queries, keys, values, valid_lens):
    """Pair batches (largest with smallest valid_len) onto 8 cores; split
    cores into two groups of 4 with their own (c0, c1) chunk shapes."""
    order = np.argsort(-valid_lens, kind="stable")
    perm = np.empty(B, dtype=np.int64)
    for c in range(NCORES):
        perm[c * BPC + 0] = order[c]
        perm[c * BPC + 1] = order[B - 1 - c]
    ktiles = np.ceil(valid_lens[perm] / 128.0).astype(np.int64).clip(min=1)

    # per-core (t0, t1); group consecutive cores (t0 desc order) while the
    # group's max(t0)+max(t1) stays at the global floor, so the binding core
    # does no more k-tiles than it must
    core_tt = [(int(ktiles[c * BPC]), int(ktiles[c * BPC + 1]))
               for c in range(NCORES)]
    floor = max(t0 + t1 for t0, t1 in core_tt)
    ccs = []          # per-group (t0, t1)
    gsizes = []       # cores per group
    cur = None
    for t0, t1 in core_tt:
        if cur is None:
            cur = (t0, t1)
            gsizes.append(1)
        else:
            m0, m1 = max(cur[0], t0), max(cur[1], t1)
            if m0 + m1 <= floor:
                cur = (m0, m1)
                gsizes[-1] += 1
            else:
                ccs.append(cur)
                cur = (t0, t1)
                gsizes.append(1)
    ccs.append(cur)

    qT = np.ascontiguousarray(queries.transpose(0, 2, 1))
    kT = np.ascontiguousarray(keys.transpose(0, 2, 1))
    v1 = np.concatenate([values, np.ones((B, N, 1), np.float32)], axis=2)
    mb = np.where(np.arange(N)[None, :] < valid_lens[:, None], 0.0, NEG).astype(np.float32)
    ones = np.ones(N, np.float32)
    iden = np.eye(D + 1, dtype=np.float32)

    in_maps = []
    for c in range(NCORES):
        idx = perm[c * BPC:(c + 1) * BPC]
        in_maps.append({
            "qT": np.ascontiguousarray(qT[idx]),
            "kT": np.ascontiguousarray(kT[idx]),
            "v1": np.ascontiguousarray(v1[idx]),
            "mb": np.ascontiguousarray(mb[idx]),
            "ones": ones,
            "iden": iden,
        })
    return perm, (ccs, gsizes), in_maps


def kernel(queries, keys, values, valid_lens):
    queries = np.ascontiguousarray(np.asarray(queries), dtype=np.float32)
    keys = np.ascontiguousarray(np.asarray(keys), dtype=np.float32)
    values = np.ascontiguousarray(np.asarray(values), dtype=np.float32)
    valid_lens = np.asarray(valid_lens).astype(np.int32)

    perm, (ccs, gsizes), in_maps = _shard(queries, keys, values, valid_lens)
    if len(ccs) == 1:
        nc = _get_nc(ccs[0])
        res = run_bass_kernel_spmd(nc, in_maps, core_ids=list(range(NCORES)))
        results = res.results
    else:
        ncs = [_get_nc(cc) for cc in ccs]
        groups = []
        k = 0
        for sz in gsizes:
            groups.append(in_maps[k:k + sz])
            k += sz
        results = _run_groups(ncs, groups)

    out = np.empty((B, N, D), np.float32)
    attn = np.empty((B, N, N), np.float32)
    for c in range(NCORES):
        for s in range(BPC):
            bi = perm[c * BPC + s]
            out[bi] = results[c]["out"][s]
            attn[bi] = results[c]["attn"][s]
    return out, attn


# revision 24
# speedup vs baseline: 1.0384x; 1.0384x over previous
"""Masked dot-product attention (B=16, Nq=Nkv=2048, d=64) on 8 TRN2 NeuronCores.

Strategy (per core: 2 batch slots, data-parallel over B):
  - Host pre-transposes Q/K to [64, N] per batch (input prep, like the mask),
    so Q^T/K^T tiles DMA straight into SBUF. Row 64 of Q^T is all-ones, row
    64 of K^T is the additive mask (0 / -1e6): the QK matmul computes masked
    scores directly in BOTH orientations (f32r, 1 cyc/row).
  - S^T tiles [k=128, q=1024] (two matmuls sharing lhsT into one 2-bank psum)
    -> one ACT exp (scale=1/8) -> E^T f32r, feeding the AV matmul
    (lhsT = V1 = V with ones column -> row 64 of out^T = softmax denominator).
  - out^T [65, 512] PE-transposed to [q, 65]; r = 1/den (DVE reciprocal);
    out = out' * r (DVE tensor_scalar).
  - attn tiles computed DIRECTLY in [q, k] layout: S = matmul(lhsT=Q^T slice,
    rhs=K^T chunk) -> ACT exp -> SBUF; DVE tensor_scalar_mul by r into attn
    row buffers -> SWDGE DMA. Only Exp on ACT (single table load).
  - Sparsity: slot0 computes t0 k-tiles of 128, slot1 t1 (valid_len rounded
    up to 128); host pairs batches (largest with smallest valid_len) and
    splits the 8 cores into two groups of 4 with their own compiled (t0, t1)
    programs, dispatched concurrently on disjoint device subsets. attn
    columns beyond the computed range stay zero (outputs are pre-zeroed
    device buffers). PE runs at a fixed 1.2 GHz on this part (HAM never
    ungates, even for bf16 streams), so the design minimizes PE cycles:
    f32r everywhere (1 cyc/row at N>=256), no on-device transposes of E.
"""
import numpy as np
import concourse.bacc as bacc
import concourse.mybir as mybir
import concourse.tile as tile
from concourse.bass_utils import run_bass_kernel_spmd
from contextlib import ExitStack

dt = mybir.dt
F32 = dt.float32
F32R = dt.float32r

B, N, D = 16, 2048, 64
NCORES = 8
BPC = B // NCORES
NEG = -1e6
SCALE = 1.0 / 8.0

_NC_CACHE = {}


def _build(ktile_counts):
    """ktile_counts: tuple of BPC ints, #k-tiles (of 128) computed per slot."""
    nc = bacc.Bacc(None, target_bir_lowering=False)
    qt_d = nc.dram_tensor("qT", [BPC, D, N], F32, kind="ExternalInput")
    kt_d = nc.dram_tensor("kT", [BPC, D, N], F32, kind="ExternalInput")
    v1_d = nc.dram_tensor("v1", [BPC, N, D + 1], F32, kind="ExternalInput")
    mb_d = nc.dram_tensor("mb", [BPC, N], F32, kind="ExternalInput")
    ones_d = nc.dram_tensor("ones", [N], F32, kind="ExternalInput")
    iden_d = nc.dram_tensor("iden", [D + 1, D + 1], F32, kind="ExternalInput")
    attn_d = nc.dram_tensor("attn", [BPC, N, N], F32, kind="ExternalOutput")
    out_d = nc.dram_tensor("out", [BPC, N, D], F32, kind="ExternalOutput")

    Exp = mybir.ActivationFunctionType.Exp
    maxt = max(ktile_counts)

    def kchunks(t):
        """split t k-tiles into chunks of <=4 tiles: [(tile0, ntiles), ...]"""
        out = []
        k = 0
        while k < t:
            n = min(4, t - k)
            out.append((k, n))
            k += n
        return out

    with tile.TileContext(nc) as tc, ExitStack() as ctx:
        const_p = ctx.enter_context(tc.tile_pool(name="const", bufs=1))
        kt_p = ctx.enter_context(tc.tile_pool(name="ktp", bufs=2 * 4))
        qt_p = ctx.enter_context(tc.tile_pool(name="qtp", bufs=2 * 2))
        v_p = ctx.enter_context(tc.tile_pool(name="vp", bufs=2))
        et_p = ctx.enter_context(tc.tile_pool(name="etp", bufs=16))
        e2_p = ctx.enter_context(tc.tile_pool(name="e2p", bufs=20))
        ot_p = ctx.enter_context(tc.tile_pool(name="otp", bufs=4))
        r_p = ctx.enter_context(tc.tile_pool(name="rp", bufs=16))
        o_p = ctx.enter_context(tc.tile_pool(name="op", bufs=4))
        arow_p = ctx.enter_context(tc.tile_pool(name="arowp", bufs=4))

        ps_s = ctx.enter_context(tc.tile_pool(name="ps_s", bufs=2, space="PSUM"))
        ps_s2 = ctx.enter_context(tc.tile_pool(name="ps_s2", bufs=2, space="PSUM"))
        ps_av = ctx.enter_context(tc.tile_pool(name="ps_av", bufs=1, space="PSUM"))
        ps_m = ctx.enter_context(tc.tile_pool(name="ps_m", bufs=1, space="PSUM"))

        iden = const_p.tile([D + 1, D + 1], F32)
        nc.sync.dma_start(iden[:], iden_d[:, :])

        def attn_exp_phase(qc, KTc, QTh, t):
            """S direct + exp -> e2 tiles for all 4 q-subtiles of chunk qc.
            Emitted BEFORE the AV chain so PE/ACT have bridge work while AV
            waits on exp1 stragglers. Returns e2 tiles for the mul phase."""
            QT = QTh[qc // 2]
            g = (qc % 2) * 512
            e2s = {}
            for qs in range(4):
                g0 = g + qs * 128
                for kc, (t0, nt) in enumerate(kchunks(t)):
                    w = 128 * nt
                    s2 = ps_s2.tile([128, 512], F32, tag="ps_s2")
                    nc.tensor.matmul(
                        s2[:, 0:w], QT[:, g0:g0 + 128], KTc[kc][:, 0:w],
                        start=True, stop=True,
                    )
                    e2 = e2_p.tile([128, 512], F32, tag="e2")
                    nc.scalar.activation(e2[:, 0:w], s2[:, 0:w], Exp, scale=SCALE)
                    e2s[(qs, kc)] = e2
            return e2s

        def attn_mul_phase(b, qc, e2s, rs, t, kw):
            """r-mul the e2 tiles into attn row buffers and DMA them out."""
            for qs in range(4):
                arow = arow_p.tile([128, 128 * maxt], F32, tag="arow")
                for kc, (t0, nt) in enumerate(kchunks(t)):
                    w = 128 * nt
                    nc.vector.tensor_scalar_mul(
                        arow[:, t0 * 128:t0 * 128 + w], e2s[(qs, kc)][:, 0:w],
                        rs[qs][:, 0:1],
                    )
                row0 = qc * 512 + qs * 128
                nc.gpsimd.dma_start(attn_d[b, row0:row0 + 128, 0:kw], arow[:, 0:kw])

        def emit_loads(b):
            t = ktile_counts[b]
            kw = 128 * t
            KTc = []
            QTh = []

            def load_kc(c):
                t0, nt = kchunks(t)[c]
                w = 128 * nt
                KT = kt_p.tile([65, 512], F32R, tag="ktc")
                nc.sync.dma_start(
                    KT[64:65, 0:w], mb_d[b, t0 * 128:t0 * 128 + w].bitcast(F32R)
                )
                nc.sync.dma_start(
                    KT[0:D, 0:w], kt_d[b, :, t0 * 128:t0 * 128 + w].bitcast(F32R)
                )
                KTc.append(KT)

            def load_qh(h):
                QT = qt_p.tile([65, 1024], F32R, tag="qth")
                nc.sync.dma_start(
                    QT[64:65, :], ones_d[h * 1024:(h + 1) * 1024].bitcast(F32R)
                )
                nc.sync.dma_start(
                    QT[0:D, :], qt_d[b, :, h * 1024:(h + 1) * 1024].bitcast(F32R)
                )
                QTh.append(QT)

            load_kc(0)
            load_qh(0)
            V1s = v_p.tile([128, maxt, D + 1], F32R, tag="v1s")
            nc.sync.dma_start(
                V1s[:, 0:t, :],
                v1_d[b, 0:kw, :].rearrange("(t p) d -> p t d", p=128).bitcast(F32R),
            )
            for c in range(1, len(kchunks(t))):
                load_kc(c)
            load_qh(1)
            return KTc, QTh, V1s

        def emit_compute(b, qp, KTc, QTh, V1s):
            t = ktile_counts[b]
            nkt = t
            kw = 128 * t
            QT = QTh[qp]
            ets = []
            for kt in range(nkt):
                sp = ps_s.tile([128, 1024], F32, tag="ps_s")
                lhs = KTc[kt // 4][:, (kt % 4) * 128:((kt % 4) + 1) * 128]
                nc.tensor.matmul(
                    sp[:, 0:512], lhs, QT[:, 0:512], start=True, stop=True
                )
                nc.tensor.matmul(
                    sp[:, 512:1024], lhs, QT[:, 512:1024], start=True, stop=True
                )
                et = et_p.tile([128, 1024], F32R, tag="et")
                nc.scalar.activation(et[:], sp[:], Exp, scale=SCALE)
                ets.append(et)

            for half in range(2):
                qc = qp * 2 + half
                h0 = half * 512
                e2s = attn_exp_phase(qc, KTc, QTh, t)
                avp = ps_av.tile([D + 1, 512], F32, tag="ps_av")
                for kt in range(nkt):
                    nc.tensor.matmul(
                        avp[:], V1s[:, kt, :], ets[kt][:, h0:h0 + 512],
                        start=(kt == 0), stop=(kt == nkt - 1),
                    )
                oT = ot_p.tile([D + 1, 512], F32, tag="ot")
                nc.vector.tensor_copy(oT[:], avp[:])

                rs = []
                for qs in range(4):
                    otp = ps_m.tile([128, D + 1], F32, tag="psm")
                    nc.tensor.transpose(
                        otp[:], oT[:, qs * 128:(qs + 1) * 128], iden[:]
                    )
                    r = r_p.tile([128, 1], F32, tag="r")
                    nc.vector.reciprocal(r[:], otp[:, D:D + 1])
                    osb = o_p.tile([128, D], F32, tag="o")
                    nc.vector.tensor_scalar_mul(osb[:], otp[:, 0:D], r[:, 0:1])
                    nc.sync.dma_start(
                        out_d[b, qc * 512 + qs * 128: qc * 512 + (qs + 1) * 128, :],
                        osb[:],
                    )
                    rs.append(r)

                attn_mul_phase(b, qc, e2s, rs, t, kw)

        l0 = emit_loads(0)
        emit_compute(0, 0, *l0)
        l1 = emit_loads(1)
        emit_compute(1, 0, *l1)
        emit_compute(0, 1, *l0)
        emit_compute(1, 1, *l1)

    nc.finalize()
    return nc


def _get_nc(chunk_counts):
    key = tuple(chunk_counts)
    if key not in _NC_CACHE:
        _NC_CACHE[key] = _build(key)
    return _NC_CACHE[key]


def _dispatch_group(nc, in_maps, devices):
    """Mirror of bass2jax.run_bass_via_pjrt's multi-core branch, but on an
    explicit device subset. Returns (lazy out_arrs, out_names, out_avals);
    caller materializes with np.asarray after dispatching all groups."""
    import jax
    from jax.experimental.shard_map import shard_map
    from jax.sharding import Mesh, PartitionSpec
    import concourse.mybir as mybir
    from concourse.bass2jax import (
        _bass_exec_p, install_neuronx_cc_hook, partition_id_tensor,
    )

    install_neuronx_cc_hook()

    partition_name = nc.partition_id_tensor.name if nc.partition_id_tensor else None
    in_names, out_names, out_avals, zero_outs = [], [], [], []
    for alloc in nc.m.functions[0].allocations:
        if not isinstance(alloc, mybir.MemoryLocationSet):
            continue
        name = alloc.memorylocations[0].name
        if alloc.kind == "ExternalInput":
            if name != partition_name:
                in_names.append(name)
        elif alloc.kind == "ExternalOutput":
            out_names.append(name)
            shape = tuple(alloc.tensor_shape)
            dtype = mybir.dt.np(alloc.dtype)
            out_avals.append(jax.core.ShapedArray(shape, dtype))
            zero_outs.append(np.zeros(shape, dtype))
    n_params = len(in_names)
    n_outs = len(out_avals)
    all_names = in_names + out_names
    if partition_name is not None:
        all_names.append(partition_name)
    donate = tuple(range(n_params, n_params + n_outs))

    def _body(*args):
        operands = list(args)
        if partition_name is not None:
            operands.append(partition_id_tensor())
        outs = _bass_exec_p.bind(
            *operands,
            out_avals=tuple(out_avals),
            in_names=tuple(all_names),
            out_names=tuple(out_names),
            lowering_input_output_aliases=(),
            sim_require_finite=True,
            sim_require_nnan=True,
            nc=nc,
        )
        return tuple(outs)

    n_cores = len(devices)
    mesh = Mesh(np.asarray(devices), ("core",))
    in_specs = (PartitionSpec("core"),) * (n_params + n_outs)
    out_specs = (PartitionSpec("core"),) * n_outs
    sharded = jax.jit(
        shard_map(_body, mesh=mesh, in_specs=in_specs, out_specs=out_specs,
                  check_rep=False),
        donate_argnums=donate, keep_unused=True,
    )
    concat_in = [
        np.concatenate([np.asarray(in_maps[c][in_names[i]]) for c in range(n_cores)],
                       axis=0)
        for i in range(n_params)
    ]
    concat_zeros = [
        np.zeros((n_cores * z.shape[0], *z.shape[1:]), z.dtype) for z in zero_outs
    ]
    out_arrs = sharded(*concat_in, *concat_zeros)
    return out_arrs, out_names, out_avals


def _run_groups(group_ncs, group_maps):
    """Dispatch each group's program on its device subset, then materialize.
    group_ncs: list of nc; group_maps: list of list-of-in_map (4 cores each)."""
    import jax
    devices = jax.devices()
    results = []
    k = 0
    lazy = []
    for nc, maps in zip(group_ncs, group_maps):
        devs = devices[k:k + len(maps)]
        lazy.append(_dispatch_group(nc, maps, devs))
        k += len(maps)
    for (out_arrs, out_names, out_avals), maps in zip(lazy, group_maps):
        n_cores = len(maps)
        per_core = [
            {name: np.asarray(out_arrs[i]).reshape(n_cores, *out_avals[i].shape)[c]
             for i, name in enumerate(out_names)}
            for c in range(n_cores)
        ]
        results.extend(per_core)
    return results


def _shard(queries, keys, values, valid_lens):
    """Pair batches (largest with smallest valid_len) onto 8 cores; split
    cores into two groups of 4 with their own (c0, c1) chunk shapes."""
    order = np.argsort(-valid_lens, kind="stable")
    perm = np.empty(B, dtype=np.int64)
    for c in range(NCORES):
        perm[c * BPC + 0] = order[c]
        perm[c * BPC + 1] = order[B - 1 - c]
    ktiles = np.ceil(valid_lens[perm] / 128.0).astype(np.int64).clip(min=1)

    # per-core (t0, t1); group consecutive cores (t0 desc order) while the
    # group's max(t0)+max(t1) stays at the global floor, so the binding core
    # does no more k-tiles than it must
    core_tt = [(int(ktiles[c * BPC]), int(ktiles[c * BPC + 1]))
               for c in range(NCORES)]
    floor = max(t0 + t1 for t0, t1 in core_tt)
    ccs = []          # per-group (t0, t1)
    gsizes = []       # cores per group
    cur = None
    for t0, t1 in core_tt:
        if cur is None:
            cur = (t0, t1)
            gsizes.append(1)
        else:
            m0, m1 = max(cur[0], t0), max(cur[1], t1)
            if m0 + m1 <= floor:
                cur = (m0, m1)
                gsizes[-1] += 1
            else:
                ccs.append(cur)
                cur = (t0, t1)
                gsizes.append(1)
    ccs.append(cur)

    qT = np.ascontiguousarray(queries.transpose(0, 2, 1))
    kT = np.ascontiguousarray(keys.transpose(0, 2, 1))
    v1 = np.concatenate([values, np.ones((B, N, 1), np.float32)], axis=2)
    mb = np.where(np.arange(N)[None, :] < valid_lens[:, None], 0.0, NEG).astype(np.float32)
    ones = np.ones(N, np.float32)
    iden = np.eye(D + 1, dtype=np.float32)

    in_maps = []
    for c in range(NCORES):
        idx = perm[c * BPC:(c + 1) * BPC]
        in_maps.append({
            "qT": np.ascontiguousarray(qT[idx]),
            "kT": np.ascontiguousarray(kT[idx]),
            "v1": np.ascontiguousarray(v1[idx]),
            "mb": np.ascontiguousarray(mb[idx]),
            "ones": ones,
            "iden": iden,
        })
    return perm, (ccs, gsizes), in_maps


def kernel(queries, keys, values, valid_lens):
    queries = np.ascontiguousarray(np.asarray(queries), dtype=np.float32)
    keys = np.ascontiguousarray(np.asarray(keys), dtype=np.float32)
    values = np.ascontiguousarray(np.asarray(values), dtype=np.float32)
    valid_lens = np.asarray(valid_lens).astype(np.int32)

    perm, (ccs, gsizes), in_maps = _shard(queries, keys, values, valid_lens)
    if len(ccs) == 1:
        nc = _get_nc(ccs[0])
        res = run_bass_kernel_spmd(nc, in_maps, core_ids=list(range(NCORES)))
        results = res.results
    else:
        ncs = [_get_nc(cc) for cc in ccs]
        groups = []
        k = 0
        for sz in gsizes:
            groups.append(in_maps[k:k + sz])
            k += sz
        results = _run_groups(ncs, groups)

    out = np.empty((B, N, D), np.float32)
    attn = np.empty((B, N, N), np.float32)
    for c in range(NCORES):
        for s in range(BPC):
            bi = perm[c * BPC + s]
            out[bi] = results[c]["out"][s]
            attn[bi] = results[c]["attn"][s]
    return out, attn


# revision 25
# speedup vs baseline: 1.0432x; 1.0046x over previous
"""Masked dot-product attention (B=16, Nq=Nkv=2048, d=64) on 8 TRN2 NeuronCores.

Strategy (per core: 2 batch slots, data-parallel over B):
  - Host pre-transposes Q/K to [64, N] per batch (input prep, like the mask),
    so Q^T/K^T tiles DMA straight into SBUF. Row 64 of Q^T is all-ones, row
    64 of K^T is the additive mask (0 / -1e6): the QK matmul computes masked
    scores directly in BOTH orientations (f32r, 1 cyc/row).
  - S^T tiles [k=128, q=1024] (two matmuls sharing lhsT into one 2-bank psum)
    -> one ACT exp (scale=1/8) -> E^T f32r, feeding the AV matmul
    (lhsT = V1 = V with ones column -> row 64 of out^T = softmax denominator).
  - out^T [65, 512] PE-transposed to [q, 65]; r = 1/den (DVE reciprocal);
    out = out' * r (DVE tensor_scalar).
  - attn tiles computed DIRECTLY in [q, k] layout: S = matmul(lhsT=Q^T slice,
    rhs=K^T chunk) -> ACT exp -> SBUF; DVE tensor_scalar_mul by r into attn
    row buffers -> SWDGE DMA. Only Exp on ACT (single table load).
  - Sparsity: slot0 computes t0 k-tiles of 128, slot1 t1 (valid_len rounded
    up to 128); host pairs batches (largest with smallest valid_len) and
    splits the 8 cores into two groups of 4 with their own compiled (t0, t1)
    programs, dispatched concurrently on disjoint device subsets. attn
    columns beyond the computed range stay zero (outputs are pre-zeroed
    device buffers). PE runs at a fixed 1.2 GHz on this part (HAM never
    ungates, even for bf16 streams), so the design minimizes PE cycles:
    f32r everywhere (1 cyc/row at N>=256), no on-device transposes of E.
"""
import numpy as np
import concourse.bacc as bacc
import concourse.mybir as mybir
import concourse.tile as tile
from concourse.bass_utils import run_bass_kernel_spmd
from contextlib import ExitStack

dt = mybir.dt
F32 = dt.float32
F32R = dt.float32r

B, N, D = 16, 2048, 64
NCORES = 8
BPC = B // NCORES
NEG = -1e6
SCALE = 1.0 / 8.0

_NC_CACHE = {}


def _build(ktile_counts):
    """ktile_counts: tuple of BPC ints, #k-tiles (of 128) computed per slot."""
    nc = bacc.Bacc(None, target_bir_lowering=False)
    qt_d = nc.dram_tensor("qT", [BPC, D, N], F32, kind="ExternalInput")
    kt_d = nc.dram_tensor("kT", [BPC, D, N], F32, kind="ExternalInput")
    v1_d = nc.dram_tensor("v1", [BPC, N, D + 1], F32, kind="ExternalInput")
    mb_d = nc.dram_tensor("mb", [BPC, N], F32, kind="ExternalInput")
    ones_d = nc.dram_tensor("ones", [N], F32, kind="ExternalInput")
    iden_d = nc.dram_tensor("iden", [D + 1, D + 1], F32, kind="ExternalInput")
    attn_d = nc.dram_tensor("attn", [BPC, N, N], F32, kind="ExternalOutput")
    out_d = nc.dram_tensor("out", [BPC, N, D], F32, kind="ExternalOutput")

    Exp = mybir.ActivationFunctionType.Exp
    maxt = max(ktile_counts)

    def kchunks(t):
        """split t k-tiles into chunks of <=4 tiles: [(tile0, ntiles), ...]"""
        out = []
        k = 0
        while k < t:
            n = min(4, t - k)
            out.append((k, n))
            k += n
        return out

    with tile.TileContext(nc) as tc, ExitStack() as ctx:
        const_p = ctx.enter_context(tc.tile_pool(name="const", bufs=1))
        kt_p = ctx.enter_context(tc.tile_pool(name="ktp", bufs=2 * 4))
        qt_p = ctx.enter_context(tc.tile_pool(name="qtp", bufs=2 * 2))
        v_p = ctx.enter_context(tc.tile_pool(name="vp", bufs=2))
        et_p = ctx.enter_context(tc.tile_pool(name="etp", bufs=16))
        e2_p = ctx.enter_context(tc.tile_pool(name="e2p", bufs=20))
        ot_p = ctx.enter_context(tc.tile_pool(name="otp", bufs=4))
        r_p = ctx.enter_context(tc.tile_pool(name="rp", bufs=16))
        o_p = ctx.enter_context(tc.tile_pool(name="op", bufs=4))
        arow_p = ctx.enter_context(tc.tile_pool(name="arowp", bufs=4))

        ps_s = ctx.enter_context(tc.tile_pool(name="ps_s", bufs=2, space="PSUM"))
        ps_s2 = ctx.enter_context(tc.tile_pool(name="ps_s2", bufs=2, space="PSUM"))
        ps_av = ctx.enter_context(tc.tile_pool(name="ps_av", bufs=1, space="PSUM"))
        ps_m = ctx.enter_context(tc.tile_pool(name="ps_m", bufs=1, space="PSUM"))

        iden = const_p.tile([D + 1, D + 1], F32)
        nc.sync.dma_start(iden[:], iden_d[:, :])

        def attn_exp_phase(qc, KTc, QTh, t):
            """S direct + exp -> e2 tiles for all 4 q-subtiles of chunk qc.
            Emitted BEFORE the AV chain so PE/ACT have bridge work while AV
            waits on exp1 stragglers. Returns e2 tiles for the mul phase."""
            QT = QTh[qc // 2]
            g = (qc % 2) * 512
            e2s = {}
            for qs in range(4):
                g0 = g + qs * 128
                for kc, (t0, nt) in enumerate(kchunks(t)):
                    w = 128 * nt
                    s2 = ps_s2.tile([128, 512], F32, tag="ps_s2")
                    nc.tensor.matmul(
                        s2[:, 0:w], QT[:, g0:g0 + 128], KTc[kc][:, 0:w],
                        start=True, stop=True,
                    )
                    e2 = e2_p.tile([128, 512], F32, tag="e2")
                    nc.scalar.activation(e2[:, 0:w], s2[:, 0:w], Exp, scale=SCALE)
                    e2s[(qs, kc)] = e2
            return e2s

        def attn_mul_phase(b, qc, e2s, rs, t, kw):
            """r-mul the e2 tiles into attn row buffers and DMA them out."""
            for qs in range(4):
                arow = arow_p.tile([128, 128 * maxt], F32, tag="arow")
                for kc, (t0, nt) in enumerate(kchunks(t)):
                    w = 128 * nt
                    nc.vector.tensor_scalar_mul(
                        arow[:, t0 * 128:t0 * 128 + w], e2s[(qs, kc)][:, 0:w],
                        rs[qs][:, 0:1],
                    )
                row0 = qc * 512 + qs * 128
                nc.sync.dma_start(attn_d[b, row0:row0 + 128, 0:kw], arow[:, 0:kw])

        def emit_loads(b):
            t = ktile_counts[b]
            kw = 128 * t
            KTc = []
            QTh = []

            def load_kc(c):
                t0, nt = kchunks(t)[c]
                w = 128 * nt
                KT = kt_p.tile([65, 512], F32R, tag="ktc")
                nc.sync.dma_start(
                    KT[64:65, 0:w], mb_d[b, t0 * 128:t0 * 128 + w].bitcast(F32R)
                )
                nc.sync.dma_start(
                    KT[0:D, 0:w], kt_d[b, :, t0 * 128:t0 * 128 + w].bitcast(F32R)
                )
                KTc.append(KT)

            def load_qh(h):
                QT = qt_p.tile([65, 1024], F32R, tag="qth")
                nc.sync.dma_start(
                    QT[64:65, :], ones_d[h * 1024:(h + 1) * 1024].bitcast(F32R)
                )
                nc.sync.dma_start(
                    QT[0:D, :], qt_d[b, :, h * 1024:(h + 1) * 1024].bitcast(F32R)
                )
                QTh.append(QT)

            load_kc(0)
            load_qh(0)
            V1s = v_p.tile([128, maxt, D + 1], F32R, tag="v1s")
            nc.sync.dma_start(
                V1s[:, 0:t, :],
                v1_d[b, 0:kw, :].rearrange("(t p) d -> p t d", p=128).bitcast(F32R),
            )
            for c in range(1, len(kchunks(t))):
                load_kc(c)
            load_qh(1)
            return KTc, QTh, V1s

        def emit_compute(b, qp, KTc, QTh, V1s):
            t = ktile_counts[b]
            nkt = t
            kw = 128 * t
            QT = QTh[qp]
            ets = []
            for kt in range(nkt):
                sp = ps_s.tile([128, 1024], F32, tag="ps_s")
                lhs = KTc[kt // 4][:, (kt % 4) * 128:((kt % 4) + 1) * 128]
                nc.tensor.matmul(
                    sp[:, 0:512], lhs, QT[:, 0:512], start=True, stop=True
                )
                nc.tensor.matmul(
                    sp[:, 512:1024], lhs, QT[:, 512:1024], start=True, stop=True
                )
                et = et_p.tile([128, 1024], F32R, tag="et")
                nc.scalar.activation(et[:], sp[:], Exp, scale=SCALE)
                ets.append(et)

            for half in range(2):
                qc = qp * 2 + half
                h0 = half * 512
                e2s = attn_exp_phase(qc, KTc, QTh, t)
                avp = ps_av.tile([D + 1, 512], F32, tag="ps_av")
                for kt in range(nkt):
                    nc.tensor.matmul(
                        avp[:], V1s[:, kt, :], ets[kt][:, h0:h0 + 512],
                        start=(kt == 0), stop=(kt == nkt - 1),
                    )
                oT = ot_p.tile([D + 1, 512], F32, tag="ot")
                nc.vector.tensor_copy(oT[:], avp[:])

                rs = []
                for qs in range(4):
                    otp = ps_m.tile([128, D + 1], F32, tag="psm")
                    nc.tensor.transpose(
                        otp[:], oT[:, qs * 128:(qs + 1) * 128], iden[:]
                    )
                    r = r_p.tile([128, 1], F32, tag="r")
                    nc.vector.reciprocal(r[:], otp[:, D:D + 1])
                    osb = o_p.tile([128, D], F32, tag="o")
                    nc.vector.tensor_scalar_mul(osb[:], otp[:, 0:D], r[:, 0:1])
                    nc.sync.dma_start(
                        out_d[b, qc * 512 + qs * 128: qc * 512 + (qs + 1) * 128, :],
                        osb[:],
                    )
                    rs.append(r)

                attn_mul_phase(b, qc, e2s, rs, t, kw)

        l0 = emit_loads(0)
        emit_compute(0, 0, *l0)
        l1 = emit_loads(1)
        emit_compute(1, 0, *l1)
        emit_compute(0, 1, *l0)
        emit_compute(1, 1, *l1)

    nc.finalize()
    return nc


def _get_nc(chunk_counts):
    key = tuple(chunk_counts)
    if key not in _NC_CACHE:
        _NC_CACHE[key] = _build(key)
    return _NC_CACHE[key]


def _dispatch_group(nc, in_maps, devices):
    """Mirror of bass2jax.run_bass_via_pjrt's multi-core branch, but on an
    explicit device subset. Returns (lazy out_arrs, out_names, out_avals);
    caller materializes with np.asarray after dispatching all groups."""
    import jax
    from jax.experimental.shard_map import shard_map
    from jax.sharding import Mesh, PartitionSpec
    import concourse.mybir as mybir
    from concourse.bass2jax import (
        _bass_exec_p, install_neuronx_cc_hook, partition_id_tensor,
    )

    install_neuronx_cc_hook()

    partition_name = nc.partition_id_tensor.name if nc.partition_id_tensor else None
    in_names, out_names, out_avals, zero_outs = [], [], [], []
    for alloc in nc.m.functions[0].allocations:
        if not isinstance(alloc, mybir.MemoryLocationSet):
            continue
        name = alloc.memorylocations[0].name
        if alloc.kind == "ExternalInput":
            if name != partition_name:
                in_names.append(name)
        elif alloc.kind == "ExternalOutput":
            out_names.append(name)
            shape = tuple(alloc.tensor_shape)
            dtype = mybir.dt.np(alloc.dtype)
            out_avals.append(jax.core.ShapedArray(shape, dtype))
            zero_outs.append(np.zeros(shape, dtype))
    n_params = len(in_names)
    n_outs = len(out_avals)
    all_names = in_names + out_names
    if partition_name is not None:
        all_names.append(partition_name)
    donate = tuple(range(n_params, n_params + n_outs))

    def _body(*args):
        operands = list(args)
        if partition_name is not None:
            operands.append(partition_id_tensor())
        outs = _bass_exec_p.bind(
            *operands,
            out_avals=tuple(out_avals),
            in_names=tuple(all_names),
            out_names=tuple(out_names),
            lowering_input_output_aliases=(),
            sim_require_finite=True,
            sim_require_nnan=True,
            nc=nc,
        )
        return tuple(outs)

    n_cores = len(devices)
    mesh = Mesh(np.asarray(devices), ("core",))
    in_specs = (PartitionSpec("core"),) * (n_params + n_outs)
    out_specs = (PartitionSpec("core"),) * n_outs
    sharded = jax.jit(
        shard_map(_body, mesh=mesh, in_specs=in_specs, out_specs=out_specs,
                  check_rep=False),
        donate_argnums=donate, keep_unused=True,
    )
    concat_in = [
        np.concatenate([np.asarray(in_maps[c][in_names[i]]) for c in range(n_cores)],
                       axis=0)
        for i in range(n_params)
    ]
    concat_zeros = [
        np.zeros((n_cores * z.shape[0], *z.shape[1:]), z.dtype) for z in zero_outs
    ]
    out_arrs = sharded(*concat_in, *concat_zeros)
    return out_arrs, out_names, out_avals


def _run_groups(group_ncs, group_maps):
    """Dispatch each group's program on its device subset, then materialize.
    group_ncs: list of nc; group_maps: list of list-of-in_map (4 cores each)."""
    import jax
    devices = jax.devices()
    results = []
    k = 0
    lazy = []
    for nc, maps in zip(group_ncs, group_maps):
        devs = devices[k:k + len(maps)]
        lazy.append(_dispatch_group(nc, maps, devs))
        k += len(maps)
    for (out_arrs, out_names, out_avals), maps in zip(lazy, group_maps):
        n_cores = len(maps)
        per_core = [
            {name: np.asarray(out_arrs[i]).reshape(n_cores, *out_avals[i].shape)[c]
             for i, name in enumerate(out_names)}
            for c in range(n_cores)
        ]
        results.extend(per_core)
    return results


def _shard(queries, keys, values, valid_lens):
    """Pair batches (largest with smallest valid_len) onto 8 cores; split
    cores into two groups of 4 with their own (c0, c1) chunk shapes."""
    order = np.argsort(-valid_lens, kind="stable")
    perm = np.empty(B, dtype=np.int64)
    for c in range(NCORES):
        perm[c * BPC + 0] = order[c]
        perm[c * BPC + 1] = order[B - 1 - c]
    ktiles = np.ceil(valid_lens[perm] / 128.0).astype(np.int64).clip(min=1)

    # per-core (t0, t1); group consecutive cores (t0 desc order) while the
    # group's max(t0)+max(t1) stays at the global floor, so the binding core
    # does no more k-tiles than it must
    core_tt = [(int(ktiles[c * BPC]), int(ktiles[c * BPC + 1]))
               for c in range(NCORES)]
    floor = max(t0 + t1 for t0, t1 in core_tt)
    ccs = []          # per-group (t0, t1)
    gsizes = []       # cores per group
    cur = None
    for t0, t1 in core_tt:
        if cur is None:
            cur = (t0, t1)
            gsizes.append(1)
        else:
            m0, m1 = max(cur[0], t0), max(cur[1], t1)
            if m0 + m1 <= floor:
                cur = (m0, m1)
                gsizes[-1] += 1
            else:
                ccs.append(cur)
                cur = (t0, t1)
                gsizes.append(1)
    ccs.append(cur)

    qT = np.ascontiguousarray(queries.transpose(0, 2, 1))
    kT = np.ascontiguousarray(keys.transpose(0, 2, 1))
    v1 = np.concatenate([values, np.ones((B, N, 1), np.float32)], axis=2)
    mb = np.where(np.arange(N)[None, :] < valid_lens[:, None], 0.0, NEG).astype(np.float32)
    ones = np.ones(N, np.float32)
    iden = np.eye(D + 1, dtype=np.float32)

    in_maps = []
    for c in range(NCORES):
        idx = perm[c * BPC:(c + 1) * BPC]
        in_maps.append({
            "qT": np.ascontiguousarray(qT[idx]),
            "kT": np.ascontiguousarray(kT[idx]),
            "v1": np.ascontiguousarray(v1[idx]),
            "mb": np.ascontiguousarray(mb[idx]),
            "ones": ones,
            "iden": iden,
        })
    return perm, (ccs, gsizes), in_maps


def kernel(queries, keys, values, valid_lens):
    queries = np.ascontiguousarray(np.asarray(queries), dtype=np.float32)
    keys = np.ascontiguousarray(np.asarray(keys), dtype=np.float32)
    values = np.ascontiguousarray(np.asarray(values), dtype=np.float32)
    valid_lens = np.asarray(valid_lens).astype(np.int32)

    perm, (ccs, gsizes), in_maps = _shard(queries, keys, values, valid_lens)
    if len(ccs) == 1:
        nc = _get_nc(ccs[0])
        res = run_bass_kernel_spmd(nc, in_maps, core_ids=list(range(NCORES)))
        results = res.results
    else:
        ncs = [_get_nc(cc) for cc in ccs]
        groups = []
        k = 0
        for sz in gsizes:
            groups.append(in_maps[k:k + sz])
            k += sz
        results = _run_groups(ncs, groups)

    out = np.empty((B, N, D), np.float32)
    attn = np.empty((B, N, N), np.float32)
    for c in range(NCORES):
        for s in range(BPC):
            bi = perm[c * BPC + s]
            out[bi] = results[c]["out"][s]
            attn[bi] = results[c]["attn"][s]
    return out, attn
